# revision 6
# baseline (speedup 1.0000x reference)
"""Trainium2 Bass kernel for nn_Block_55448027791422 (dense transformer block).

Strategy: pure data-parallel over batch B=16 across 8 NeuronCores (2 batches
per core), zero collectives.  All activations live on-chip in a channel-major
(C on partitions, tokens on free dim) layout so every matmul contracts over
the partition dim with no on-device transposes; weights are pre-transposed and
cast to bf16 on the host.  LayerNorm per-token stats are computed with
all-ones stationary matmuls (which replicate the sums over all 128 output
partitions for free), softmax runs without max-subtraction (scores are bounded
|s| < 7 for this problem), the rel_bias add is folded into a bf16 multiply by
host-precomputed exp(rel_bias), and softmax denominators come from a ones
column appended to V.  Per-channel biases/gains are folded into weights or
applied as per-partition scalars through the ScalarEngine activation path.
"""

import os
import sys
import contextlib
import ctypes
import types

import numpy as np

for _p in ("/opt/trn_rl_repo",):
    if _p not in sys.path:
        sys.path.insert(0, _p)

import ml_dtypes

bfloat16 = ml_dtypes.bfloat16

# ---------------------------------------------------------------- constants
B, N, C, H, HD, HID, TXT = 16, 616, 768, 12, 64, 3072, 40
NCORES = 8
BL = B // NCORES            # 2 batches per core
NT = BL * N                 # 1232 tokens per core
KT = C // 128               # 6 channel tiles
MT_QK = (2 * C) // 128      # 12 output tiles for q,k
KT_HID = HID // 128         # 24 hidden tiles
EPS = 1e-5
KEY_TILES = [(0, 128), (128, 128), (256, 128), (384, 128), (512, 104)]
NKT = len(KEY_TILES)
SCALE = HD ** -0.5


def _chunks(total, step=512, base=0):
    out, o = [], 0
    while o < total:
        s = min(step, total - o)
        out.append((base + o, s))
        o += s
    return out


NT_CH = _chunks(NT)                      # [(0,512),(512,512),(1024,208)]
def _bch(b):                             # per-batch query chunks
    return _chunks(N, 512, b * N)


# const-vector column indexes in the [128, NCONST] consts tensor
def _cc():
    idx = {}
    c = 0
    for name, n in [("bqk", MT_QK), ("g1", KT), ("g1bp", KT),
                    ("g2", KT), ("g2bt2", KT), ("g2bi2", KT),
                    ("bt1", KT_HID), ("bi1", KT_HID)]:
        idx[name] = c
        c += n
    return idx, c


CCOL, NCONST = _cc()

_ENV_TRACE = "BASS_KERNEL_TRACE"
LAST_EXEC_TIME_NS = None
LAST_TRACE_PATH = None


# ------------------------------------------------------- axon profile hook
def _install_ntff_hook():
    """run_bass_kernel_spmd(trace=True) under axon needs antenv.axon_hooks."""
    if "antenv.axon_hooks" in sys.modules:
        return
    so_path = "/opt/axon/libaxon_pjrt.so"
    state = {"h": None}

    def _build():
        try:
            lib = ctypes.CDLL(so_path)
        except OSError:
            return None
        if not hasattr(lib, "axon_start_nrt_profile"):
            return None
        lib.axon_start_nrt_profile.argtypes = [ctypes.POINTER(ctypes.c_int64),
                                               ctypes.c_size_t]
        lib.axon_start_nrt_profile.restype = ctypes.c_int64
        lib.axon_stop_nrt_profile.argtypes = [ctypes.c_char_p]
        lib.axon_stop_nrt_profile.restype = ctypes.c_int64

        @contextlib.contextmanager
        def _hook(output_dir, device_ids):
            import jax
            jax.devices()
            if device_ids:
                ids = (ctypes.c_int64 * len(device_ids))(*device_ids)
                rc = lib.axon_start_nrt_profile(ids, len(device_ids))
            else:
                rc = lib.axon_start_nrt_profile(None, 0)
            if rc != 0:
                raise RuntimeError(f"axon_start_nrt_profile rc={rc}")
            try:
                yield
            finally:
                n = lib.axon_stop_nrt_profile(str(output_dir).encode())
                if n < 0:
                    raise RuntimeError(f"axon_stop_nrt_profile rc={n}")

        return _hook

    def get_axon_ntff_profile_hook():
        if state["h"] is None:
            state["h"] = _build()
        return state["h"]

    mod = types.ModuleType("antenv.axon_hooks")
    mod.get_axon_ntff_profile_hook = get_axon_ntff_profile_hook
    mod.set_axon_ntff_profile_hook = lambda h: state.update(h=h)
    sys.modules["antenv.axon_hooks"] = mod


# ------------------------------------------------------------ graph builder
_NC_CACHE = {}


def _build_nc():
    import concourse.bass as bass  # noqa: F401
    import concourse.mybir as mybir
    import concourse.tile as tile
    from concourse import bacc

    F32 = mybir.dt.float32
    BF16 = mybir.dt.bfloat16
    AF = mybir.ActivationFunctionType
    ALU = mybir.AluOpType

    nc = bacc.Bacc(None, target_bir_lowering=False)
    d = nc.declare_dram_parameter
    x_d = d("x", [BL, C, N], F32, isOutput=False)
    eb_d = d("expbias", [H, NKT, 128, N], BF16, isOutput=False)
    wqk_d = d("wqk", [C, 2 * C], BF16, isOutput=False)
    wv_d = d("wv", [C, C], BF16, isOutput=False)
    wproj_d = d("wproj", [C, C], BF16, isOutput=False)
    wt1_d = d("wt1", [C, HID], BF16, isOutput=False)
    wi1_d = d("wi1", [C, HID], BF16, isOutput=False)
    wt2_d = d("wt2", [HID, C], BF16, isOutput=False)
    wi2_d = d("wi2", [HID, C], BF16, isOutput=False)
    consts_d = d("consts", [128, NCONST], F32, isOutput=False)
    out_d = d("out", [BL, C, N], F32, isOutput=True)

    with tile.TileContext(nc) as tc:
        with contextlib.ExitStack() as octx:
            per = octx.enter_context(tc.tile_pool(name="perm", bufs=1))
            # persistent tiles
            consts = per.tile([128, NCONST], F32, tag="consts")

            def cvec(name, i):
                return consts[:, CCOL[name] + i:CCOL[name] + i + 1]

            nc.sync.dma_start(out=consts[:], in_=consts_d[:])
            ones128 = per.tile([128, 128], BF16, tag="ones128")
            nc.vector.memset(ones128[:], 1.0)
            eps_ap = per.tile([128, 1], F32, tag="epsap")
            nc.vector.memset(eps_ap[:], EPS)

            x_sb = [per.tile([128, NT], F32, tag="x", bufs=KT, name=f"x{i}")
                    for i in range(KT)]
            for kt in range(KT):
                for b in range(BL):
                    nc.sync.dma_start(
                        out=x_sb[kt][:, b * N:(b + 1) * N],
                        in_=x_d[b, kt * 128:(kt + 1) * 128, :])

            o_sb = [per.tile([128, NT], BF16, tag="o", bufs=KT, name=f"o{i}")
                    for i in range(KT)]
            v_sb = [[per.tile([128, H * 65], BF16, tag="v", bufs=BL * NKT,
                              name=f"v{b}_{i}")
                     for i in range(NKT)] for b in range(BL)]

            # n ring: first 6 allocs = x_bf (bf16 cast of x), next 6 = n,
            # then 6 = n2 (slot reuse is ordered by true dataflow)
            def n_ring(idx):
                return per.tile([128, NT], BF16, tag="nring", bufs=KT,
                                name=f"nring{idx}")

            # ---------------- early pool (qkv / attention / proj / stats)
            with contextlib.ExitStack() as ectx:
                ep = ectx.enter_context(tc.tile_pool(name="early", bufs=1))
                psA = ectx.enter_context(
                    tc.tile_pool(name="psA", bufs=1, space="PSUM"))

                wqk_sb = [ep.tile([128, 2 * C], BF16, tag="wqk", bufs=KT,
                                  name=f"wqk{i}") for i in range(KT)]
                wv_sb = [ep.tile([128, C], BF16, tag="wv", bufs=KT,
                                 name=f"wv{i}") for i in range(KT)]
                wproj_sb = [ep.tile([128, C], BF16, tag="wproj", bufs=KT,
                                    name=f"wpj{i}") for i in range(KT)]
                for kt in range(KT):
                    sl = slice(kt * 128, (kt + 1) * 128)
                    nc.sync.dma_start(out=wqk_sb[kt][:], in_=wqk_d[sl, :])
                    nc.sync.dma_start(out=wv_sb[kt][:], in_=wv_d[sl, :])
                    nc.sync.dma_start(out=wproj_sb[kt][:], in_=wproj_d[sl, :])

                qk_sb = [ep.tile([128, NT], BF16, tag="qk", bufs=MT_QK,
                                 name=f"qk{i}") for i in range(MT_QK)]

                # ---------------------------------------------- LayerNorm
                def layer_norm(x_tiles, n_tiles_out):
                    """Emit per-token LN of C-major x_tiles -> bf16 n tiles
                    (no affine - gains/biases are folded into weights)."""
                    mean_sb, r_sb = [], []
                    for (o, w) in NT_CH:
                        ps_m = psA.tile([128, 616], F32, tag="big2", bufs=2,
                                        name="psm")[:, :w]
                        ps_e = psA.tile([128, 616], F32, tag="big2", bufs=2,
                                        name="pse")[:, :w]
                        for kt in range(KT):
                            xbt = ep.tile([128, 512], BF16, tag="xbt",
                                          bufs=4, name="xbt")[:, :w]
                            sqt = ep.tile([128, 512], BF16, tag="sqt",
                                          bufs=4, name="sqt")[:, :w]
                            nc.vector.tensor_copy(xbt, x_tiles[kt][:, o:o + w])
                            nc.scalar.activation(sqt, x_tiles[kt][:, o:o + w],
                                                 AF.Square)
                            nc.tensor.matmul(ps_m, ones128[:], xbt,
                                             start=(kt == 0),
                                             stop=(kt == KT - 1))
                            nc.tensor.matmul(ps_e, ones128[:], sqt,
                                             start=(kt == 0),
                                             stop=(kt == KT - 1))
                        m2 = ep.tile([128, 512], F32, tag="lntmp", bufs=3,
                                     name="m2")[:, :w]
                        # m2 = (sum/sqrt(C))^2 = C * mean^2
                        nc.scalar.activation(m2, ps_m, AF.Square,
                                             scale=float(C ** -0.5))
                        dd = ep.tile([128, 512], F32, tag="lntmp", bufs=3,
                                     name="dd")[:, :w]
                        nc.vector.tensor_sub(dd, ps_e, m2)
                        s = ep.tile([128, 512], F32, tag="lntmp", bufs=3,
                                    name="s")[:, :w]
                        nc.scalar.activation(s, dd, AF.Sqrt,
                                             bias=eps_ap[:, 0:1],
                                             scale=float(1.0 / C))
                        r = ep.tile([128, 512], F32, tag="lnr", bufs=6,
                                    name="r")[:, :w]
                        nc.vector.reciprocal_approx_fast(out=r, in_=s)
                        mn = ep.tile([128, 512], F32, tag="lnr", bufs=6,
                                     name="mn")[:, :w]
                        nc.scalar.mul(mn, ps_m, float(1.0 / C))
                        mean_sb.append(mn)
                        r_sb.append(r)
                    for kt in range(KT):
                        for ci, (o, w) in enumerate(NT_CH):
                            t = ep.tile([128, 512], F32, tag="lnt", bufs=2,
                                        name="t")[:, :w]
                            nc.gpsimd.tensor_sub(t, x_tiles[kt][:, o:o + w],
                                                 mean_sb[ci])
                            nc.vector.tensor_mul(
                                n_tiles_out[kt][:, o:o + w], t, r_sb[ci])

                n_sb = [n_ring(f"n{i}") for i in range(KT)]
                layer_norm(x_sb, n_sb)

                # ------------------------------------------------ q,k matmul
                for mt in range(MT_QK):
                    msl = slice(mt * 128, (mt + 1) * 128)
                    pss = []
                    for (o, w) in NT_CH:
                        ps = psA.tile([128, 616], F32, tag="big2", bufs=2,
                                      name="qkps")[:, :w]
                        pss.append((ps, o, w))
                        for kt in range(KT):
                            nc.tensor.matmul(ps, wqk_sb[kt][:, msl],
                                             n_sb[kt][:, o:o + w],
                                             start=(kt == 0),
                                             stop=(kt == KT - 1))
                    for ps, o, w in pss:
                        nc.scalar.activation(qk_sb[mt][:, o:o + w], ps,
                                             AF.Identity,
                                             bias=cvec("bqk", mt), scale=1.0)

                # ------------------------------------------------- v matmul
                # out: per (b, key-tile) [tok<=128, 12 heads * 65] with a
                # ones column per head at position 65h+64 (softmax denom)
                for b in range(BL):
                    for ktl, (koff, ksz) in enumerate(KEY_TILES):
                        toff = b * N + koff
                        vt = v_sb[b][ktl]
                        nc.vector.memset(vt[:], 0.0)
                        vt3 = vt.rearrange("p (h e) -> p h e", e=65)
                        nc.vector.memset(vt3[:, :, 64:65], 1.0)
                        for (o, w) in ((0, 512), (512, 256)):
                            ps = psA.tile([128, 616], F32, tag="big2", bufs=2,
                                          name="vps")[:ksz, :w]
                            for kt in range(KT):
                                nc.tensor.matmul(
                                    ps, n_sb[kt][:, toff:toff + ksz],
                                    wv_sb[kt][:, o:o + w],
                                    start=(kt == 0), stop=(kt == KT - 1))
                            # strided copy: psum [ksz, 8*64] -> v_t heads
                            nheads = w // 64
                            h0 = o // 64
                            nc.scalar.copy(
                                vt3[:ksz, h0:h0 + nheads, 0:64],
                                ps.rearrange("p (h e) -> p h e", e=64))

                # ------------------------------------------------ attention
                eb_pool = [None] * (H * NKT)

                def eb_tile(h, ktl):
                    i = h * NKT + ktl
                    if eb_pool[i] is None:
                        t = ep.tile([128, N], BF16, tag="eb", bufs=9,
                                    name=f"eb{i}")
                        nc.sync.dma_start(out=t[:], in_=eb_d[h, ktl])
                        eb_pool[i] = t
                    return eb_pool[i]

                for h in range(H):
                    qt = qk_sb[h // 2]
                    kt_t = qk_sb[KT + h // 2]
                    po = (h % 2) * 64
                    for b in range(BL):
                        praw = []
                        for ktl, (koff, ksz) in enumerate(KEY_TILES):
                            pr = ep.tile([128, N], BF16, tag="probs",
                                         bufs=11, name=f"pr{ktl}")
                            praw.append(pr)
                            ps = psA.tile([128, N], F32, tag="big2",
                                          bufs=2, name="scps")[:ksz, :]
                            for (qo, qw) in _bch(b):
                                nc.tensor.matmul(
                                    ps[:, qo - b * N:qo - b * N + qw],
                                    kt_t[po:po + 64,
                                         b * N + koff:b * N + koff + ksz],
                                    qt[po:po + 64, qo:qo + qw],
                                    start=True, stop=True)
                            nc.scalar.activation(pr[:ksz, :], ps, AF.Exp)
                            eng = nc.vector if ktl % 2 == 0 else nc.gpsimd
                            eng.tensor_mul(pr[:ksz, :], pr[:ksz, :],
                                           eb_tile(h, ktl)[:ksz, :])
                        # attn @ v (+ones row -> denominators in row 64)
                        vcol = slice(h * 65, (h + 1) * 65)
                        pv = psA.tile([128, N], F32, tag="acc",
                                      bufs=2, name="po")[:65, :]
                        for ktl, (koff, ksz) in enumerate(KEY_TILES):
                            for (qo, qw) in _bch(b):
                                nc.tensor.matmul(
                                    pv[:, qo - b * N:qo - b * N + qw],
                                    v_sb[b][ktl][:ksz, vcol],
                                    praw[ktl][:ksz, qo - b * N:
                                              qo - b * N + qw],
                                    start=(ktl == 0), stop=(ktl == NKT - 1))
                        den = ep.tile([1, N], F32, tag="den", bufs=4,
                                      name="den")
                        nc.scalar.copy(den[0:1, :], pv[64:65, :])
                        rec = ep.tile([1, N], F32, tag="den", bufs=4,
                                      name="rec")
                        nc.vector.reciprocal_approx_fast(out=rec, in_=den)
                        recb = ep.tile([64, N], F32, tag="recb", bufs=2,
                                       name="recb")
                        nc.gpsimd.partition_broadcast(recb[:], rec[0:1, :])
                        ot = o_sb[h // 2]
                        nc.vector.tensor_mul(
                            ot[po:po + 64, b * N:(b + 1) * N],
                            pv[0:64, :], recb[:])

                # ------------------------------------------------ proj (+res)
                for b in range(BL):
                    for mt in range(KT):
                        msl = slice(mt * 128, (mt + 1) * 128)
                        pss = []
                        for (qo, qw) in _bch(b):
                            ps = psA.tile([128, 616], F32, tag="big2", bufs=2,
                                          name="pjps")[:, :qw]
                            pss.append((ps, qo, qw))
                            for kt in range(KT):
                                nc.tensor.matmul(ps, wproj_sb[kt][:, msl],
                                                 o_sb[kt][:, qo:qo + qw],
                                                 start=(kt == 0),
                                                 stop=(kt == KT - 1))
                        for ps, qo, qw in pss:
                            xs = x_sb[mt][:, qo:qo + qw]
                            nc.vector.affine_then_add(
                                xs, ps, xs, scale=cvec("g1", mt),
                                bias=cvec("g1bp", mt))

                # ---------------------------------------------- LN2 -> n2
                n2_sb = [n_ring(f"m{i}") for i in range(KT)]
                layer_norm(x_sb, n2_sb)

            # -------------------------------------------------- MLP phase
            with contextlib.ExitStack() as mctx:
                mp = mctx.enter_context(tc.tile_pool(name="mlp", bufs=1))
                psB = mctx.enter_context(
                    tc.tile_pool(name="psB", bufs=1, space="PSUM"))

                h_sb = [mp.tile([128, NT], BF16, tag="h", bufs=KT_HID,
                                name=f"h{i}") for i in range(KT_HID)]

                def w1_tiles(src):
                    ts = [mp.tile([128, HID], BF16, tag="w1", bufs=KT,
                                  name=f"w1_{i}") for i in range(KT)]
                    for kt in range(KT):
                        nc.sync.dma_start(
                            out=ts[kt][:],
                            in_=src[kt * 128:(kt + 1) * 128, :])
                    return ts

                def w2_tiles(src):
                    ts = [mp.tile([128, C], BF16, tag="w2", bufs=KT_HID,
                                  name=f"w2_{i}") for i in range(KT_HID)]
                    for kt in range(KT_HID):
                        nc.sync.dma_start(
                            out=ts[kt][:],
                            in_=src[kt * 128:(kt + 1) * 128, :])
                    return ts

                # chunk sets: text = 40 tokens per batch, image = 576
                text_ch = [(b * N, TXT) for b in range(BL)]
                img_ch = []
                for b in range(BL):
                    img_ch += _chunks(N - TXT, 512, b * N + TXT)

                def mlp1(w1sb, cols, bias_name):
                    for mt in range(KT_HID):
                        msl = slice(mt * 128, (mt + 1) * 128)
                        pss = []
                        for (o, w) in cols:
                            ps = psB.tile([128, 512], F32, tag="macc",
                                          bufs=6, name="m1ps")[:, :w]
                            pss.append((ps, o, w))
                            for kt in range(KT):
                                nc.tensor.matmul(ps, w1sb[kt][:, msl],
                                                 n2_sb[kt][:, o:o + w],
                                                 start=(kt == 0),
                                                 stop=(kt == KT - 1))
                        for ps, o, w in pss:
                            nc.scalar.activation(h_sb[mt][:, o:o + w], ps,
                                                 AF.Gelu,
                                                 bias=cvec(bias_name, mt),
                                                 scale=1.0)

                def mlp2(w2sb, cols, bias_name):
                    for mt in range(KT):
                        msl = slice(mt * 128, (mt + 1) * 128)
                        pss = []
                        for (o, w) in cols:
                            ps = psB.tile([128, 512], F32, tag="macc",
                                          bufs=6, name="m2ps")[:, :w]
                            pss.append((ps, o, w))
                            for kt in range(KT_HID):
                                nc.tensor.matmul(ps, w2sb[kt][:, msl],
                                                 h_sb[kt][:, o:o + w],
                                                 start=(kt == 0),
                                                 stop=(kt == KT_HID - 1))
                        for ps, o, w in pss:
                            xs = x_sb[mt][:, o:o + w]
                            nc.vector.affine_then_add(
                                xs, ps, xs, scale=cvec("g2", mt),
                                bias=cvec(bias_name, mt))

                wt1_sb = w1_tiles(wt1_d)
                mlp1(wt1_sb, text_ch, "bt1")
                wt2_sb = w2_tiles(wt2_d)
                mlp2(wt2_sb, text_ch, "g2bt2")
                wi1_sb = w1_tiles(wi1_d)
                mlp1(wi1_sb, img_ch, "bi1")
                wi2_sb = w2_tiles(wi2_d)
                mlp2(wi2_sb, img_ch, "g2bi2")

                # --------------------------------------------------- output
                for b in range(BL):
                    for mt in range(KT):
                        nc.sync.dma_start(
                            out=out_d[b, mt * 128:(mt + 1) * 128, :],
                            in_=x_sb[mt][:, b * N:(b + 1) * N])

    nc.compile()
    return nc


# ---------------------------------------------------------- host-side prep
def _prep_inputs(inputs):
    f = lambda k: np.asarray(inputs[k], dtype=np.float32)
    x = f("x")
    rel_bias = f("rel_bias")
    w_qkv = f("w_qkv")
    ln1_g, ln1_b = f("ln1_g"), f("ln1_b")
    q_bias, v_bias = f("q_bias"), f("v_bias")
    w_proj, b_proj = f("w_proj"), f("b_proj")
    gamma1, gamma2 = f("gamma1"), f("gamma2")

    Wq = w_qkv[:C] * ln1_g[None, :]
    Wk = w_qkv[C:2 * C] * ln1_g[None, :]
    Wv = w_qkv[2 * C:] * ln1_g[None, :]
    bq = q_bias + w_qkv[:C] @ ln1_b
    bk = w_qkv[C:2 * C] @ ln1_b
    bv = v_bias + w_qkv[2 * C:] @ ln1_b
    Wq *= SCALE
    bq *= SCALE
    b_projp = b_proj + w_proj @ bv

    wqk = np.concatenate([Wq, Wk], axis=0).T          # [C, 1536]
    wv = Wv.T                                         # [C, C]
    wproj = w_proj.T                                  # [C, C]

    def mlp_fold(w1, b1, w2, b2, g, bb):
        w1f = w1 * g[None, :]
        b1f = b1 + w1 @ bb
        return w1f.T, b1f, w2.T, b2

    wt1, bt1, wt2, bt2 = mlp_fold(f("wt1"), f("bt1"), f("wt2"), f("bt2"),
                                  f("ln2t_g"), f("ln2t_b"))
    wi1, bi1, wi2, bi2 = mlp_fold(f("wi1"), f("bi1"), f("wi2"), f("bi2"),
                                  f("ln2i_g"), f("ln2i_b"))

    consts = np.zeros((128, NCONST), np.float32)
    bqk = np.concatenate([bq, bk])
    for i in range(MT_QK):
        consts[:, CCOL["bqk"] + i] = bqk[i * 128:(i + 1) * 128]
    for i in range(KT):
        sl = slice(i * 128, (i + 1) * 128)
        consts[:, CCOL["g1"] + i] = gamma1[sl]
        consts[:, CCOL["g1bp"] + i] = (gamma1 * b_projp)[sl]
        consts[:, CCOL["g2"] + i] = gamma2[sl]
        consts[:, CCOL["g2bt2"] + i] = (gamma2 * bt2)[sl]
        consts[:, CCOL["g2bi2"] + i] = (gamma2 * bi2)[sl]
    for i in range(KT_HID):
        sl = slice(i * 128, (i + 1) * 128)
        consts[:, CCOL["bt1"] + i] = bt1[sl]
        consts[:, CCOL["bi1"] + i] = bi1[sl]

    # exp(rel_bias) transposed to [H, key, query], keys padded to 640
    ebt = np.exp(rel_bias).transpose(0, 2, 1)
    eb = np.zeros((H, NKT * 128, N), np.float32)
    eb[:, :N, :] = ebt
    eb = eb.reshape(H, NKT, 128, N)

    bf = lambda a: np.ascontiguousarray(a, dtype=np.float32).astype(bfloat16)
    shared = {
        "expbias": bf(eb),
        "wqk": bf(wqk), "wv": bf(wv), "wproj": bf(wproj),
        "wt1": bf(wt1), "wi1": bf(wi1), "wt2": bf(wt2), "wi2": bf(wi2),
        "consts": np.ascontiguousarray(consts),
    }
    # per-core x shards, channel-major
    xs = x.reshape(NCORES, BL, N, C).transpose(0, 1, 3, 2)
    in_maps = []
    for c in range(NCORES):
        m = dict(shared)
        m["x"] = np.ascontiguousarray(xs[c])
        in_maps.append(m)
    return in_maps


def kernel(**inputs):
    global LAST_EXEC_TIME_NS, LAST_TRACE_PATH
    _install_ntff_hook()
    from concourse.bass_utils import run_bass_kernel_spmd

    if "nc" not in _NC_CACHE:
        _NC_CACHE["nc"] = _build_nc()
    nc = _NC_CACHE["nc"]

    in_maps = _prep_inputs(inputs)
    trace = os.environ.get(_ENV_TRACE, "") == "1"
    res = run_bass_kernel_spmd(nc, in_maps, core_ids=list(range(NCORES)),
                               trace=trace)
    LAST_EXEC_TIME_NS = res.exec_time_ns
    if trace and res.instructions_and_trace is not None:
        LAST_TRACE_PATH = res.instructions_and_trace[1]

    out = np.empty((B, N, C), np.float32)
    for c in range(NCORES):
        oc = np.asarray(res.results[c]["out"])          # [BL, C, N]
        out[c * BL:(c + 1) * BL] = oc.transpose(0, 2, 1)
    return out


# revision 7
# speedup vs baseline: 1.6220x; 1.6220x over previous
"""Trainium2 Bass kernel for nn_Block_55448027791422 (dense transformer block).

Strategy: pure data-parallel over batch B=16 across 8 NeuronCores (2 batches
per core), zero collectives.  All activations live on-chip in a channel-major
(C on partitions, tokens on free dim) layout so every matmul contracts over
the partition dim with no on-device transposes; weights are pre-transposed and
cast to bf16 on the host.  LayerNorm per-token stats are computed with
all-ones stationary matmuls (which replicate the sums over all 128 output
partitions for free), softmax runs without max-subtraction (scores are bounded
|s| < 7 for this problem), the rel_bias add is folded into a bf16 multiply by
host-precomputed exp(rel_bias), and softmax denominators come from a ones
column appended to V.  Per-channel biases/gains are folded into weights or
applied as per-partition scalars through the ScalarEngine activation path.
"""

import os
import sys
import contextlib
import ctypes
import types

import numpy as np

for _p in ("/opt/trn_rl_repo",):
    if _p not in sys.path:
        sys.path.insert(0, _p)

import ml_dtypes

bfloat16 = ml_dtypes.bfloat16

# ---------------------------------------------------------------- constants
B, N, C, H, HD, HID, TXT = 16, 616, 768, 12, 64, 3072, 40
NCORES = 8
BL = B // NCORES            # 2 batches per core
NT = BL * N                 # 1232 tokens per core
KT = C // 128               # 6 channel tiles
MT_QK = (2 * C) // 128      # 12 output tiles for q,k
KT_HID = HID // 128         # 24 hidden tiles
EPS = 1e-5
KEY_TILES = [(0, 128), (128, 128), (256, 128), (384, 128), (512, 104)]
NKT = len(KEY_TILES)
SCALE = HD ** -0.5


def _chunks(total, step=512, base=0):
    out, o = [], 0
    while o < total:
        s = min(step, total - o)
        out.append((base + o, s))
        o += s
    return out


NT_CH = _chunks(NT)                      # [(0,512),(512,512),(1024,208)]
def _bch(b):                             # per-batch query chunks
    return _chunks(N, 512, b * N)


# const-vector column indexes in the [128, NCONST] consts tensor
def _cc():
    idx = {}
    c = 0
    for name, n in [("bqk", MT_QK), ("g1", KT), ("g1bp", KT),
                    ("g2", KT), ("g2bt2", KT), ("g2bi2", KT),
                    ("bt1", KT_HID), ("bi1", KT_HID)]:
        idx[name] = c
        c += n
    return idx, c


CCOL, NCONST = _cc()

_ENV_TRACE = "BASS_KERNEL_TRACE"
LAST_EXEC_TIME_NS = None
LAST_TRACE_PATH = None


# ------------------------------------------------------- axon profile hook
def _install_ntff_hook():
    """run_bass_kernel_spmd(trace=True) under axon needs antenv.axon_hooks."""
    if "antenv.axon_hooks" in sys.modules:
        return
    so_path = "/opt/axon/libaxon_pjrt.so"
    state = {"h": None}

    def _build():
        try:
            lib = ctypes.CDLL(so_path)
        except OSError:
            return None
        if not hasattr(lib, "axon_start_nrt_profile"):
            return None
        lib.axon_start_nrt_profile.argtypes = [ctypes.POINTER(ctypes.c_int64),
                                               ctypes.c_size_t]
        lib.axon_start_nrt_profile.restype = ctypes.c_int64
        lib.axon_stop_nrt_profile.argtypes = [ctypes.c_char_p]
        lib.axon_stop_nrt_profile.restype = ctypes.c_int64

        @contextlib.contextmanager
        def _hook(output_dir, device_ids):
            import jax
            jax.devices()
            if device_ids:
                ids = (ctypes.c_int64 * len(device_ids))(*device_ids)
                rc = lib.axon_start_nrt_profile(ids, len(device_ids))
            else:
                rc = lib.axon_start_nrt_profile(None, 0)
            if rc != 0:
                raise RuntimeError(f"axon_start_nrt_profile rc={rc}")
            try:
                yield
            finally:
                n = lib.axon_stop_nrt_profile(str(output_dir).encode())
                if n < 0:
                    raise RuntimeError(f"axon_stop_nrt_profile rc={n}")

        return _hook

    def get_axon_ntff_profile_hook():
        if state["h"] is None:
            state["h"] = _build()
        return state["h"]

    mod = types.ModuleType("antenv.axon_hooks")
    mod.get_axon_ntff_profile_hook = get_axon_ntff_profile_hook
    mod.set_axon_ntff_profile_hook = lambda h: state.update(h=h)
    sys.modules["antenv.axon_hooks"] = mod


# ------------------------------------------------------------ graph builder
_NC_CACHE = {}


def _build_nc():
    import concourse.bass as bass  # noqa: F401
    import concourse.mybir as mybir
    import concourse.tile as tile
    from concourse import bacc

    F32 = mybir.dt.float32
    BF16 = mybir.dt.bfloat16
    AF = mybir.ActivationFunctionType
    ALU = mybir.AluOpType

    nc = bacc.Bacc(None, target_bir_lowering=False)
    d = nc.declare_dram_parameter
    x_d = d("x", [BL, C, N], F32, isOutput=False)
    eb_d = d("expbias", [H, NKT, 128, N], BF16, isOutput=False)
    wqk_d = d("wqk", [C, 2 * C], BF16, isOutput=False)
    wv_d = d("wv", [C, C], BF16, isOutput=False)
    wproj_d = d("wproj", [C, C], BF16, isOutput=False)
    wt1_d = d("wt1", [C, HID], BF16, isOutput=False)
    wi1_d = d("wi1", [C, HID], BF16, isOutput=False)
    wt2_d = d("wt2", [HID, C], BF16, isOutput=False)
    wi2_d = d("wi2", [HID, C], BF16, isOutput=False)
    consts_d = d("consts", [128, NCONST], F32, isOutput=False)
    out_d = d("out", [BL, C, N], F32, isOutput=True)

    with tile.TileContext(nc) as tc:
        with contextlib.ExitStack() as octx:
            per = octx.enter_context(tc.tile_pool(name="perm", bufs=1))
            # persistent tiles
            consts = per.tile([128, NCONST], F32, tag="consts")

            def cvec(name, i):
                return consts[:, CCOL[name] + i:CCOL[name] + i + 1]

            nc.sync.dma_start(out=consts[:], in_=consts_d[:])
            ones128 = per.tile([128, 128], BF16, tag="ones128")
            nc.vector.memset(ones128[:], 1.0)
            eps_ap = per.tile([128, 1], F32, tag="epsap")
            nc.vector.memset(eps_ap[:], EPS)

            x_sb = [per.tile([128, NT], F32, tag="x", bufs=KT, name=f"x{i}")
                    for i in range(KT)]
            for kt in range(KT):
                for b in range(BL):
                    nc.sync.dma_start(
                        out=x_sb[kt][:, b * N:(b + 1) * N],
                        in_=x_d[b, kt * 128:(kt + 1) * 128, :])

            o_sb = [per.tile([128, NT], BF16, tag="o", bufs=KT, name=f"o{i}")
                    for i in range(KT)]
            v_sb = [[per.tile([128, H * 65], BF16, tag="v", bufs=BL * NKT,
                              name=f"v{b}_{i}")
                     for i in range(NKT)] for b in range(BL)]

            # n ring: first 6 allocs = x_bf (bf16 cast of x), next 6 = n,
            # then 6 = n2 (slot reuse is ordered by true dataflow)
            def n_ring(idx):
                return per.tile([128, NT], BF16, tag="nring", bufs=KT,
                                name=f"nring{idx}")

            # ---------------- early pool (qkv / attention / proj / stats)
            with contextlib.ExitStack() as ectx:
                ep = ectx.enter_context(tc.tile_pool(name="early", bufs=1))
                psA = ectx.enter_context(
                    tc.tile_pool(name="psA", bufs=1, space="PSUM"))

                wqk_sb = [ep.tile([128, 2 * C], BF16, tag="wqk", bufs=KT,
                                  name=f"wqk{i}") for i in range(KT)]
                wv_sb = [ep.tile([128, C], BF16, tag="wv", bufs=KT,
                                 name=f"wv{i}") for i in range(KT)]
                wproj_sb = [ep.tile([128, C], BF16, tag="wproj", bufs=KT,
                                    name=f"wpj{i}") for i in range(KT)]
                for kt in range(KT):
                    sl = slice(kt * 128, (kt + 1) * 128)
                    nc.sync.dma_start(out=wqk_sb[kt][:], in_=wqk_d[sl, :])
                    nc.sync.dma_start(out=wv_sb[kt][:], in_=wv_d[sl, :])
                    nc.sync.dma_start(out=wproj_sb[kt][:], in_=wproj_d[sl, :])

                qk_sb = [ep.tile([128, NT], BF16, tag="qk", bufs=MT_QK,
                                 name=f"qk{i}") for i in range(MT_QK)]

                # ---------------------------------------------- LayerNorm
                def layer_norm(x_tiles, n_tiles_out):
                    """Emit per-token LN of C-major x_tiles -> bf16 n tiles
                    (no affine - gains/biases are folded into weights)."""
                    mean_sb, r_sb = [], []
                    for (o, w) in NT_CH:
                        ps_m = psA.tile([128, 616], F32, tag="big2", bufs=2,
                                        name="psm")[:, :w]
                        ps_e = psA.tile([128, 616], F32, tag="big2", bufs=2,
                                        name="pse")[:, :w]
                        for kt in range(KT):
                            xbt = ep.tile([128, 512], BF16, tag="xbt",
                                          bufs=4, name="xbt")[:, :w]
                            sqt = ep.tile([128, 512], BF16, tag="sqt",
                                          bufs=4, name="sqt")[:, :w]
                            nc.vector.tensor_copy(xbt, x_tiles[kt][:, o:o + w])
                            nc.scalar.activation(sqt, x_tiles[kt][:, o:o + w],
                                                 AF.Square)
                            nc.tensor.matmul(ps_m, ones128[:], xbt,
                                             start=(kt == 0),
                                             stop=(kt == KT - 1))
                            nc.tensor.matmul(ps_e, ones128[:], sqt,
                                             start=(kt == 0),
                                             stop=(kt == KT - 1))
                        m2 = ep.tile([128, 512], F32, tag="lntmp", bufs=3,
                                     name="m2")[:, :w]
                        # m2 = (sum/sqrt(C))^2 = C * mean^2
                        nc.scalar.activation(m2, ps_m, AF.Square,
                                             scale=float(C ** -0.5))
                        dd = ep.tile([128, 512], F32, tag="lntmp", bufs=3,
                                     name="dd")[:, :w]
                        nc.vector.tensor_sub(dd, ps_e, m2)
                        s = ep.tile([128, 512], F32, tag="lntmp", bufs=3,
                                    name="s")[:, :w]
                        nc.scalar.activation(s, dd, AF.Sqrt,
                                             bias=eps_ap[:, 0:1],
                                             scale=float(1.0 / C))
                        r = ep.tile([128, 512], F32, tag="lnr", bufs=6,
                                    name="r")[:, :w]
                        nc.vector.reciprocal_approx_fast(out=r, in_=s)
                        mn = ep.tile([128, 512], F32, tag="lnr", bufs=6,
                                     name="mn")[:, :w]
                        nc.scalar.mul(mn, ps_m, float(1.0 / C))
                        mean_sb.append(mn)
                        r_sb.append(r)
                    for kt in range(KT):
                        for ci, (o, w) in enumerate(NT_CH):
                            t = ep.tile([128, 512], F32, tag="lnt", bufs=2,
                                        name="t")[:, :w]
                            nc.gpsimd.tensor_sub(t, x_tiles[kt][:, o:o + w],
                                                 mean_sb[ci])
                            nc.vector.tensor_mul(
                                n_tiles_out[kt][:, o:o + w], t, r_sb[ci])

                n_sb = [n_ring(f"n{i}") for i in range(KT)]
                layer_norm(x_sb, n_sb)

                # ------------------------------------------------- v matmul
                # out: per (b, key-tile) [tok<=128, 12 heads * 65] with a
                # ones column per head at position 65h+64 (softmax denom)
                for b in range(BL):
                    for ktl, (koff, ksz) in enumerate(KEY_TILES):
                        toff = b * N + koff
                        vt = v_sb[b][ktl]
                        nc.vector.memset(vt[:], 0.0)
                        vt3 = vt.rearrange("p (h e) -> p h e", e=65)
                        nc.vector.memset(vt3[:, :, 64:65], 1.0)
                        for (o, w) in ((0, 512), (512, 256)):
                            ps = psA.tile([128, 616], F32, tag="big2", bufs=2,
                                          name="vps")[:ksz, :w]
                            for kt in range(KT):
                                nc.tensor.matmul(
                                    ps, n_sb[kt][:, toff:toff + ksz],
                                    wv_sb[kt][:, o:o + w],
                                    start=(kt == 0), stop=(kt == KT - 1))
                            # strided copy: psum [ksz, 8*64] -> v_t heads
                            nheads = w // 64
                            h0 = o // 64
                            nc.scalar.copy(
                                vt3[:ksz, h0:h0 + nheads, 0:64],
                                ps.rearrange("p (h e) -> p h e", e=64))

                # ------------------------------------------------ q,k matmul
                for mt in [0, 6, 1, 7, 2, 8, 3, 9, 4, 10, 5, 11]:
                    msl = slice(mt * 128, (mt + 1) * 128)
                    pss = []
                    for (o, w) in NT_CH:
                        ps = psA.tile([128, 616], F32, tag="big2", bufs=2,
                                      name="qkps")[:, :w]
                        pss.append((ps, o, w))
                        for kt in range(KT):
                            nc.tensor.matmul(ps, wqk_sb[kt][:, msl],
                                             n_sb[kt][:, o:o + w],
                                             start=(kt == 0),
                                             stop=(kt == KT - 1))
                    for ps, o, w in pss:
                        nc.scalar.activation(qk_sb[mt][:, o:o + w], ps,
                                             AF.Identity,
                                             bias=cvec("bqk", mt), scale=1.0)

                # ------------------------------------------------ attention
                eb_pool = [None] * (H * NKT)

                def eb_tile(h, ktl):
                    i = h * NKT + ktl
                    if eb_pool[i] is None:
                        t = ep.tile([128, N], BF16, tag="eb", bufs=9,
                                    name=f"eb{i}")
                        nc.sync.dma_start(out=t[:], in_=eb_d[h, ktl])
                        eb_pool[i] = t
                    return eb_pool[i]

                for h in range(H):
                    qt = qk_sb[h // 2]
                    kt_t = qk_sb[KT + h // 2]
                    po = (h % 2) * 64
                    for b in range(BL):
                        praw = []
                        for ktl, (koff, ksz) in enumerate(KEY_TILES):
                            pr = ep.tile([128, N], BF16, tag="probs",
                                         bufs=11, name=f"pr{ktl}")
                            praw.append(pr)
                            ps = psA.tile([128, N], F32, tag="big2",
                                          bufs=2, name="scps")[:ksz, :]
                            for (qo, qw) in _bch(b):
                                nc.tensor.matmul(
                                    ps[:, qo - b * N:qo - b * N + qw],
                                    kt_t[po:po + 64,
                                         b * N + koff:b * N + koff + ksz],
                                    qt[po:po + 64, qo:qo + qw],
                                    start=True, stop=True)
                            nc.scalar.activation(pr[:ksz, :], ps, AF.Exp)
                            nc.vector.tensor_mul(pr[:ksz, :], pr[:ksz, :],
                                                 eb_tile(h, ktl)[:ksz, :])
                        # attn @ v (+ones row -> denominators in row 64)
                        vcol = slice(h * 65, (h + 1) * 65)
                        pv = psA.tile([128, N], F32, tag="acc",
                                      bufs=2, name="po")[:65, :]
                        for ktl, (koff, ksz) in enumerate(KEY_TILES):
                            for (qo, qw) in _bch(b):
                                nc.tensor.matmul(
                                    pv[:, qo - b * N:qo - b * N + qw],
                                    v_sb[b][ktl][:ksz, vcol],
                                    praw[ktl][:ksz, qo - b * N:
                                              qo - b * N + qw],
                                    start=(ktl == 0), stop=(ktl == NKT - 1))
                        den = ep.tile([1, N], F32, tag="den", bufs=4,
                                      name="den")
                        nc.scalar.copy(den[0:1, :], pv[64:65, :])
                        rec = ep.tile([1, N], F32, tag="den", bufs=4,
                                      name="rec")
                        nc.vector.reciprocal_approx_fast(out=rec, in_=den)
                        recb = ep.tile([64, N], F32, tag="recb", bufs=2,
                                       name="recb")
                        nc.gpsimd.partition_broadcast(recb[:], rec[0:1, :])
                        ot = o_sb[h // 2]
                        nc.vector.tensor_mul(
                            ot[po:po + 64, b * N:(b + 1) * N],
                            pv[0:64, :], recb[:])

                # ------------------------------------------------ proj (+res)
                for b in range(BL):
                    for mt in range(KT):
                        msl = slice(mt * 128, (mt + 1) * 128)
                        pss = []
                        for (qo, qw) in _bch(b):
                            ps = psA.tile([128, 616], F32, tag="big2", bufs=2,
                                          name="pjps")[:, :qw]
                            pss.append((ps, qo, qw))
                            for kt in range(KT):
                                nc.tensor.matmul(ps, wproj_sb[kt][:, msl],
                                                 o_sb[kt][:, qo:qo + qw],
                                                 start=(kt == 0),
                                                 stop=(kt == KT - 1))
                        for ps, qo, qw in pss:
                            xs = x_sb[mt][:, qo:qo + qw]
                            nc.vector.affine_then_add(
                                xs, ps, xs, scale=cvec("g1", mt),
                                bias=cvec("g1bp", mt))

                # ---------------------------------------------- LN2 -> n2
                n2_sb = [n_ring(f"m{i}") for i in range(KT)]
                layer_norm(x_sb, n2_sb)

            # -------------------------------------------------- MLP phase
            with contextlib.ExitStack() as mctx:
                mp = mctx.enter_context(tc.tile_pool(name="mlp", bufs=1))
                psB = mctx.enter_context(
                    tc.tile_pool(name="psB", bufs=1, space="PSUM"))

                h_sb = [mp.tile([128, NT], BF16, tag="h", bufs=KT_HID,
                                name=f"h{i}") for i in range(KT_HID)]

                def w1_tiles(src):
                    ts = [mp.tile([128, HID], BF16, tag="w1", bufs=KT,
                                  name=f"w1_{i}") for i in range(KT)]
                    for kt in range(KT):
                        nc.sync.dma_start(
                            out=ts[kt][:],
                            in_=src[kt * 128:(kt + 1) * 128, :])
                    return ts

                def w2_tiles(src):
                    ts = [mp.tile([128, C], BF16, tag="w2", bufs=KT_HID,
                                  name=f"w2_{i}") for i in range(KT_HID)]
                    for kt in range(KT_HID):
                        nc.sync.dma_start(
                            out=ts[kt][:],
                            in_=src[kt * 128:(kt + 1) * 128, :])
                    return ts

                # chunk sets: text = 40 tokens per batch, image = 576
                text_ch = [(b * N, TXT) for b in range(BL)]
                img_ch = []
                for b in range(BL):
                    img_ch += _chunks(N - TXT, 512, b * N + TXT)

                def mlp1(w1sb, cols, bias_name):
                    for mt in range(KT_HID):
                        msl = slice(mt * 128, (mt + 1) * 128)
                        pss = []
                        for (o, w) in cols:
                            ps = psB.tile([128, 512], F32, tag="macc",
                                          bufs=6, name="m1ps")[:, :w]
                            pss.append((ps, o, w))
                            for kt in range(KT):
                                nc.tensor.matmul(ps, w1sb[kt][:, msl],
                                                 n2_sb[kt][:, o:o + w],
                                                 start=(kt == 0),
                                                 stop=(kt == KT - 1))
                        for ps, o, w in pss:
                            nc.scalar.activation(h_sb[mt][:, o:o + w], ps,
                                                 AF.Gelu,
                                                 bias=cvec(bias_name, mt),
                                                 scale=1.0)

                def mlp2(w2sb, cols, bias_name):
                    for mt in range(KT):
                        msl = slice(mt * 128, (mt + 1) * 128)
                        pss = []
                        for (o, w) in cols:
                            ps = psB.tile([128, 512], F32, tag="macc",
                                          bufs=6, name="m2ps")[:, :w]
                            pss.append((ps, o, w))
                            for kt in range(KT_HID):
                                nc.tensor.matmul(ps, w2sb[kt][:, msl],
                                                 h_sb[kt][:, o:o + w],
                                                 start=(kt == 0),
                                                 stop=(kt == KT_HID - 1))
                        for ps, o, w in pss:
                            xs = x_sb[mt][:, o:o + w]
                            nc.vector.affine_then_add(
                                xs, ps, xs, scale=cvec("g2", mt),
                                bias=cvec(bias_name, mt))

                wt1_sb = w1_tiles(wt1_d)
                wt2_sb = w2_tiles(wt2_d)
                mlp1(wt1_sb, text_ch, "bt1")
                mlp2(wt2_sb, text_ch, "g2bt2")
                wi1_sb = w1_tiles(wi1_d)
                mlp1(wi1_sb, img_ch, "bi1")
                wi2_sb = w2_tiles(wi2_d)
                mlp2(wi2_sb, img_ch, "g2bi2")

                # --------------------------------------------------- output
                for b in range(BL):
                    for mt in range(KT):
                        nc.sync.dma_start(
                            out=out_d[b, mt * 128:(mt + 1) * 128, :],
                            in_=x_sb[mt][:, b * N:(b + 1) * N])

    nc.compile()
    return nc


# ---------------------------------------------------------- host-side prep
def _prep_inputs(inputs):
    f = lambda k: np.asarray(inputs[k], dtype=np.float32)
    x = f("x")
    rel_bias = f("rel_bias")
    w_qkv = f("w_qkv")
    ln1_g, ln1_b = f("ln1_g"), f("ln1_b")
    q_bias, v_bias = f("q_bias"), f("v_bias")
    w_proj, b_proj = f("w_proj"), f("b_proj")
    gamma1, gamma2 = f("gamma1"), f("gamma2")

    Wq = w_qkv[:C] * ln1_g[None, :]
    Wk = w_qkv[C:2 * C] * ln1_g[None, :]
    Wv = w_qkv[2 * C:] * ln1_g[None, :]
    bq = q_bias + w_qkv[:C] @ ln1_b
    bk = w_qkv[C:2 * C] @ ln1_b
    bv = v_bias + w_qkv[2 * C:] @ ln1_b
    Wq *= SCALE
    bq *= SCALE
    b_projp = b_proj + w_proj @ bv

    wqk = np.concatenate([Wq, Wk], axis=0).T          # [C, 1536]
    wv = Wv.T                                         # [C, C]
    wproj = w_proj.T                                  # [C, C]

    def mlp_fold(w1, b1, w2, b2, g, bb):
        w1f = w1 * g[None, :]
        b1f = b1 + w1 @ bb
        return w1f.T, b1f, w2.T, b2

    wt1, bt1, wt2, bt2 = mlp_fold(f("wt1"), f("bt1"), f("wt2"), f("bt2"),
                                  f("ln2t_g"), f("ln2t_b"))
    wi1, bi1, wi2, bi2 = mlp_fold(f("wi1"), f("bi1"), f("wi2"), f("bi2"),
                                  f("ln2i_g"), f("ln2i_b"))

    consts = np.zeros((128, NCONST), np.float32)
    bqk = np.concatenate([bq, bk])
    for i in range(MT_QK):
        consts[:, CCOL["bqk"] + i] = bqk[i * 128:(i + 1) * 128]
    for i in range(KT):
        sl = slice(i * 128, (i + 1) * 128)
        consts[:, CCOL["g1"] + i] = gamma1[sl]
        consts[:, CCOL["g1bp"] + i] = (gamma1 * b_projp)[sl]
        consts[:, CCOL["g2"] + i] = gamma2[sl]
        consts[:, CCOL["g2bt2"] + i] = (gamma2 * bt2)[sl]
        consts[:, CCOL["g2bi2"] + i] = (gamma2 * bi2)[sl]
    for i in range(KT_HID):
        sl = slice(i * 128, (i + 1) * 128)
        consts[:, CCOL["bt1"] + i] = bt1[sl]
        consts[:, CCOL["bi1"] + i] = bi1[sl]

    # exp(rel_bias) transposed to [H, key, query], keys padded to 640
    ebt = np.exp(rel_bias).transpose(0, 2, 1)
    eb = np.zeros((H, NKT * 128, N), np.float32)
    eb[:, :N, :] = ebt
    eb = eb.reshape(H, NKT, 128, N)

    bf = lambda a: np.ascontiguousarray(a, dtype=np.float32).astype(bfloat16)
    shared = {
        "expbias": bf(eb),
        "wqk": bf(wqk), "wv": bf(wv), "wproj": bf(wproj),
        "wt1": bf(wt1), "wi1": bf(wi1), "wt2": bf(wt2), "wi2": bf(wi2),
        "consts": np.ascontiguousarray(consts),
    }
    # per-core x shards, channel-major
    xs = x.reshape(NCORES, BL, N, C).transpose(0, 1, 3, 2)
    in_maps = []
    for c in range(NCORES):
        m = dict(shared)
        m["x"] = np.ascontiguousarray(xs[c])
        in_maps.append(m)
    return in_maps


def kernel(**inputs):
    global LAST_EXEC_TIME_NS, LAST_TRACE_PATH
    _install_ntff_hook()
    from concourse.bass_utils import run_bass_kernel_spmd

    if "nc" not in _NC_CACHE:
        _NC_CACHE["nc"] = _build_nc()
    nc = _NC_CACHE["nc"]

    in_maps = _prep_inputs(inputs)
    trace = os.environ.get(_ENV_TRACE, "") == "1"
    res = run_bass_kernel_spmd(nc, in_maps, core_ids=list(range(NCORES)),
                               trace=trace)
    LAST_EXEC_TIME_NS = res.exec_time_ns
    if trace and res.instructions_and_trace is not None:
        LAST_TRACE_PATH = res.instructions_and_trace[1]

    out = np.empty((B, N, C), np.float32)
    for c in range(NCORES):
        oc = np.asarray(res.results[c]["out"])          # [BL, C, N]
        out[c * BL:(c + 1) * BL] = oc.transpose(0, 2, 1)
    return out
